# revision 10
# baseline (speedup 1.0000x reference)
"""Blockwise 8x8 2D DCT (ortho DCT-II) on Trainium2, 8 NeuronCores data-parallel.

Per 8x8 block: Y = A @ X @ A.T, with M = kron(I_16, A) acting on 128-row tiles.

Key trick ("fused" op): a regular PE matmul with the DATA as the stationary
operand computes  out = chunk^T @ M^T = (M @ chunk)^T  — one DCT pass plus a
128x128 transpose in a single instruction. Two fused passes give
  pass1: (M X)^T   (W-major)     pass2: ((M X) M^T)  (back to H-major)
In bf16 the stationary load gets FWL (2x), so each fused op is ~LDW+128 cols.

Modes:
  fused_bf16  : cast x->bf16 at DMA load (gpsimd SWDGE cast); both passes fused bf16.
  hybrid      : V-pass as f32r streaming matmul (x stays fp32-exact), bf16
                transposes, fused bf16 H-pass.
  stream_f32r : f32r streaming matmuls + f32r PE transposes both directions.
"""

import numpy as np
import ml_dtypes

import concourse.bass as bass
import concourse.bacc as bacc
import concourse.mybir as mybir
from concourse import tile
from concourse.bass_utils import run_bass_kernel_spmd

F32 = mybir.dt.float32
F32R = mybir.dt.float32r
BF16 = mybir.dt.bfloat16
P = 128
BLOCK = 8
N_CORES = 8

FULL_N, FULL_C, FULL_H, FULL_W = 64, 1, 1024, 1024

MODE = "fused_bf16"


class _CopyBalancer:
    """Deterministically split PSUM->SBUF copies between DVE and ACT."""

    def __init__(self, nc, dve_of_8=5):
        self.nc = nc
        self.k = dve_of_8
        self.i = 0

    def copy(self, out, in_):
        if self.i % 8 < self.k:
            self.nc.vector.tensor_copy(out, in_)
        else:
            self.nc.scalar.copy(out, in_)
        self.i += 1


def build_fused_bf16(n_img: int, img_h: int, width: int):
    rows = n_img * img_h
    nrt, nwt = img_h // P, width // P
    assert nrt % 4 == 0 and nwt % 4 == 0

    nc = bacc.Bacc("TRN2", target_bir_lowering=False, debug=False)
    x_d = nc.declare_dram_parameter("x", [rows, width], F32, isOutput=False)
    mtb_d = nc.declare_dram_parameter("mtb", [P, P], BF16, isOutput=False)
    out_d = nc.declare_dram_parameter("out", [rows, width], F32, isOutput=True)

    with tile.TileContext(nc) as tc:
        with (
            tc.tile_pool(name="consts", bufs=1) as cpool,
            tc.tile_pool(name="xin", bufs=nrt + 4) as xpool,
            tc.tile_pool(name="y1t", bufs=nwt + 4) as y1tpool,
            tc.tile_pool(name="outp", bufs=4) as outpool,
            tc.tile_pool(name="psA", bufs=4, space="PSUM") as psA,
            tc.tile_pool(name="psB", bufs=4, space="PSUM") as psB,
        ):
            cb = _CopyBalancer(nc)
            mtb_sb = cpool.tile([P, P], BF16)
            nc.sync.dma_start(mtb_sb[:], mtb_d[:])

            for img in range(n_img):
                r0 = img * img_h

                xts = []
                for rt in range(nrt):
                    xt = xpool.tile([P, width], BF16)
                    # SWDGE cast f32 -> bf16 during the load
                    nc.gpsimd.dma_start(
                        xt[:], x_d[r0 + rt * P : r0 + (rt + 1) * P, :]
                    )
                    xts.append(xt)

                # pass 1: y1t[wt][:, rt*128:+128] = (M @ x_chunk)^T
                y1ts = []
                for wt in range(nwt):
                    y1t = y1tpool.tile([P, img_h], BF16)
                    for half in range(nrt // 4):
                        ps = psA.tile([P, 512], F32)
                        for q in range(4):
                            rt = half * 4 + q
                            nc.tensor.matmul(
                                ps[:, q * P : (q + 1) * P],
                                xts[rt][:, wt * P : (wt + 1) * P],
                                mtb_sb[:],
                            )
                        cb.copy(y1t[:, half * 512 : (half + 1) * 512], ps[:])
                    y1ts.append(y1t)

                # pass 2: out[rt][:, wt*128:+128] = (M @ y1t_chunk)^T = final
                for rt in range(nrt):
                    out_sb = outpool.tile([P, width], F32)
                    for half in range(nwt // 4):
                        ps = psB.tile([P, 512], F32)
                        for q in range(4):
                            wt = half * 4 + q
                            nc.tensor.matmul(
                                ps[:, q * P : (q + 1) * P],
                                y1ts[wt][:, rt * P : (rt + 1) * P],
                                mtb_sb[:],
                            )
                        cb.copy(out_sb[:, half * 512 : (half + 1) * 512], ps[:])
                    nc.sync.dma_start(
                        out_d[r0 + rt * P : r0 + (rt + 1) * P, :], out_sb[:]
                    )

    nc.compile()
    return nc


def build_hybrid(n_img: int, img_h: int, width: int):
    rows = n_img * img_h
    nrt, nwt = img_h // P, width // P
    assert nrt % 4 == 0 and nwt % 4 == 0
    MMW = 512

    nc = bacc.Bacc("TRN2", target_bir_lowering=False, debug=False)
    x_d = nc.declare_dram_parameter("x", [rows, width], F32R, isOutput=False)
    mt_d = nc.declare_dram_parameter("mt", [P, P], F32R, isOutput=False)
    mtb_d = nc.declare_dram_parameter("mtb", [P, P], BF16, isOutput=False)
    identb_d = nc.declare_dram_parameter("identb", [P, P], BF16, isOutput=False)
    out_d = nc.declare_dram_parameter("out", [rows, width], F32, isOutput=True)

    with tile.TileContext(nc) as tc:
        with (
            tc.tile_pool(name="consts", bufs=1) as cpool,
            tc.tile_pool(name="xin", bufs=6) as xpool,
            tc.tile_pool(name="y1", bufs=nrt + 2) as y1pool,
            tc.tile_pool(name="y1t", bufs=nwt + 4) as y1tpool,
            tc.tile_pool(name="outp", bufs=4) as outpool,
            tc.tile_pool(name="psV", bufs=3, space="PSUM") as psV,
            tc.tile_pool(name="psT", bufs=3, space="PSUM") as psT,
            tc.tile_pool(name="psH", bufs=2, space="PSUM") as psH,
        ):
            cb = _CopyBalancer(nc)
            mt_sb = cpool.tile([P, P], F32R)
            mtb_sb = cpool.tile([P, P], BF16)
            identb = cpool.tile([P, P], BF16)
            nc.sync.dma_start(mt_sb[:], mt_d[:])
            nc.sync.dma_start(mtb_sb[:], mtb_d[:])
            nc.sync.dma_start(identb[:], identb_d[:])

            for img in range(n_img):
                r0 = img * img_h

                # V-pass: f32r stream, round to bf16 on the PSUM->SBUF copy
                y1s = []
                for rt in range(nrt):
                    xt = xpool.tile([P, width], F32R)
                    nc.sync.dma_start(
                        xt[:], x_d[r0 + rt * P : r0 + (rt + 1) * P, :]
                    )
                    y1 = y1pool.tile([P, width], BF16)
                    for c in range(width // MMW):
                        ps = psV.tile([P, MMW], F32)
                        nc.tensor.matmul(
                            ps[:], mt_sb[:], xt[:, c * MMW : (c + 1) * MMW]
                        )
                        cb.copy(y1[:, c * MMW : (c + 1) * MMW], ps[:])
                    y1s.append(y1)

                # T-pass: bf16 PE transposes, 8 per PSUM bank
                y1ts = []
                for wt in range(nwt):
                    y1t = y1tpool.tile([P, img_h], BF16)
                    pst = psT.tile([P, img_h], BF16)
                    for rt in range(nrt):
                        nc.tensor.transpose(
                            pst[:, rt * P : (rt + 1) * P],
                            y1s[rt][:, wt * P : (wt + 1) * P],
                            identb[:],
                        )
                    cb.copy(y1t[:], pst[:])
                    y1ts.append(y1t)

                # fused H-pass: out chunk = (y1t_chunk)^T @ M^T  (H-major)
                for rt in range(nrt):
                    out_sb = outpool.tile([P, width], F32)
                    for half in range(nwt // 4):
                        ps = psH.tile([P, 512], F32)
                        for q in range(4):
                            wt = half * 4 + q
                            nc.tensor.matmul(
                                ps[:, q * P : (q + 1) * P],
                                y1ts[wt][:, rt * P : (rt + 1) * P],
                                mtb_sb[:],
                            )
                        cb.copy(out_sb[:, half * 512 : (half + 1) * 512], ps[:])
                    nc.sync.dma_start(
                        out_d[r0 + rt * P : r0 + (rt + 1) * P, :], out_sb[:]
                    )

    nc.compile()
    return nc


def build_nc(n_img, img_h, width, mode=MODE):
    if mode == "fused_bf16":
        return build_fused_bf16(n_img, img_h, width)
    if mode == "hybrid":
        return build_hybrid(n_img, img_h, width)
    raise ValueError(mode)


def make_mt(A: np.ndarray) -> np.ndarray:
    """M^T where M = kron(I_{128/8}, A)."""
    M = np.kron(np.eye(P // BLOCK, dtype=np.float32), A.astype(np.float32))
    return np.ascontiguousarray(M.T)


def make_inputs(mode, x_core, A):
    mt = make_mt(A)
    if mode == "fused_bf16":
        return {"x": x_core, "mtb": mt.astype(ml_dtypes.bfloat16)}
    if mode == "hybrid":
        return {
            "x": x_core,
            "mt": mt,
            "mtb": mt.astype(ml_dtypes.bfloat16),
            "identb": np.eye(P, dtype=ml_dtypes.bfloat16),
        }
    raise ValueError(mode)


_NC_CACHE = {}


def _get_nc(key, *args, **kwargs):
    if key not in _NC_CACHE:
        _NC_CACHE[key] = build_nc(*args, **kwargs)
    return _NC_CACHE[key]


def kernel(x: np.ndarray, A: np.ndarray) -> np.ndarray:
    x = np.asarray(x, dtype=np.float32)
    A = np.asarray(A, dtype=np.float32)
    N, C, H, W = x.shape
    assert (N, C, H, W) == (FULL_N, FULL_C, FULL_H, FULL_W), x.shape
    per = N // N_CORES

    nc = _get_nc(("full", MODE), per * C, H, W, MODE)

    in_maps = [
        make_inputs(
            MODE,
            np.ascontiguousarray(x[c * per : (c + 1) * per].reshape(per * C * H, W)),
            A,
        )
        for c in range(N_CORES)
    ]
    res = run_bass_kernel_spmd(nc, in_maps, list(range(N_CORES)))
    outs = [
        res.results[c]["out"].reshape(per, C, H, W) for c in range(N_CORES)
    ]
    return np.concatenate(outs, axis=0)


# revision 11
# speedup vs baseline: 1.1220x; 1.1220x over previous
"""Blockwise 8x8 2D DCT (ortho DCT-II) on Trainium2, 8 NeuronCores data-parallel.

Per 8x8 block: Y = A @ X @ A.T, with M = kron(I_16, A) acting on 128-row tiles.

Key trick ("fused" op): a regular PE matmul with the DATA as the stationary
operand computes  out = chunk^T @ M^T = (M @ chunk)^T  — one DCT pass plus a
128x128 transpose in a single instruction. Two fused passes give
  pass1: (M X)^T   (W-major)     pass2: ((M X) M^T)  (back to H-major)
In bf16 the stationary load gets FWL (2x), so each fused op is ~LDW+128 cols.

Modes:
  fused_bf16  : cast x->bf16 at DMA load (gpsimd SWDGE cast); both passes fused bf16.
  hybrid      : V-pass as f32r streaming matmul (x stays fp32-exact), bf16
                transposes, fused bf16 H-pass.
  stream_f32r : f32r streaming matmuls + f32r PE transposes both directions.
"""

import numpy as np
import ml_dtypes

import concourse.bass as bass
import concourse.bacc as bacc
import concourse.mybir as mybir
from concourse import tile
from concourse.bass_utils import run_bass_kernel_spmd

F32 = mybir.dt.float32
F32R = mybir.dt.float32r
BF16 = mybir.dt.bfloat16
FP16 = mybir.dt.float16
P = 128
BLOCK = 8
N_CORES = 8

FULL_N, FULL_C, FULL_H, FULL_W = 64, 1, 1024, 1024

MODE = "fused_fp16"


class _CopyBalancer:
    """Deterministically split PSUM->SBUF copies between DVE and ACT."""

    def __init__(self, nc, dve_of_8=5):
        self.nc = nc
        self.k = dve_of_8
        self.i = 0

    def copy(self, out, in_):
        if self.i % 8 < self.k:
            self.nc.vector.tensor_copy(out, in_)
        else:
            self.nc.scalar.copy(out, in_)
        self.i += 1


def build_fused_bf16(n_img: int, img_h: int, width: int, dt16=BF16):
    rows = n_img * img_h
    nrt, nwt = img_h // P, width // P
    assert nrt % 4 == 0 and nwt % 4 == 0

    nc = bacc.Bacc("TRN2", target_bir_lowering=False, debug=False)
    x_d = nc.declare_dram_parameter("x", [rows, width], F32, isOutput=False)
    mtb_d = nc.declare_dram_parameter("mtb", [P, P], dt16, isOutput=False)
    out_d = nc.declare_dram_parameter("out", [rows, width], F32, isOutput=True)

    with tile.TileContext(nc) as tc:
        with (
            tc.tile_pool(name="consts", bufs=1) as cpool,
            tc.tile_pool(name="xin", bufs=nrt + 4) as xpool,
            tc.tile_pool(name="y1t", bufs=nwt + 4) as y1tpool,
            tc.tile_pool(name="outp", bufs=4) as outpool,
            tc.tile_pool(name="psA", bufs=4, space="PSUM") as psA,
            tc.tile_pool(name="psB", bufs=4, space="PSUM") as psB,
        ):
            cb = _CopyBalancer(nc)
            mtb_sb = cpool.tile([P, P], dt16)
            nc.sync.dma_start(mtb_sb[:], mtb_d[:])

            for img in range(n_img):
                r0 = img * img_h

                xts = []
                for rt in range(nrt):
                    xt = xpool.tile([P, width], dt16)
                    # SWDGE cast f32 -> bf16 during the load
                    nc.gpsimd.dma_start(
                        xt[:], x_d[r0 + rt * P : r0 + (rt + 1) * P, :]
                    )
                    xts.append(xt)

                # pass 1: y1t[wt][:, rt*128:+128] = (M @ x_chunk)^T
                y1ts = []
                for wt in range(nwt):
                    y1t = y1tpool.tile([P, img_h], dt16)
                    for half in range(nrt // 4):
                        ps = psA.tile([P, 512], F32)
                        for q in range(4):
                            rt = half * 4 + q
                            nc.tensor.matmul(
                                ps[:, q * P : (q + 1) * P],
                                xts[rt][:, wt * P : (wt + 1) * P],
                                mtb_sb[:],
                            )
                        cb.copy(y1t[:, half * 512 : (half + 1) * 512], ps[:])
                    y1ts.append(y1t)

                # pass 2: out[rt][:, wt*128:+128] = (M @ y1t_chunk)^T = final
                for rt in range(nrt):
                    out_sb = outpool.tile([P, width], F32)
                    for half in range(nwt // 4):
                        ps = psB.tile([P, 512], F32)
                        for q in range(4):
                            wt = half * 4 + q
                            nc.tensor.matmul(
                                ps[:, q * P : (q + 1) * P],
                                y1ts[wt][:, rt * P : (rt + 1) * P],
                                mtb_sb[:],
                            )
                        cb.copy(out_sb[:, half * 512 : (half + 1) * 512], ps[:])
                    nc.sync.dma_start(
                        out_d[r0 + rt * P : r0 + (rt + 1) * P, :], out_sb[:]
                    )

    nc.compile()
    return nc


def build_hybrid(n_img: int, img_h: int, width: int):
    rows = n_img * img_h
    nrt, nwt = img_h // P, width // P
    assert nrt % 4 == 0 and nwt % 4 == 0
    MMW = 512

    nc = bacc.Bacc("TRN2", target_bir_lowering=False, debug=False)
    x_d = nc.declare_dram_parameter("x", [rows, width], F32R, isOutput=False)
    mt_d = nc.declare_dram_parameter("mt", [P, P], F32R, isOutput=False)
    mtb_d = nc.declare_dram_parameter("mtb", [P, P], BF16, isOutput=False)
    identb_d = nc.declare_dram_parameter("identb", [P, P], BF16, isOutput=False)
    out_d = nc.declare_dram_parameter("out", [rows, width], F32, isOutput=True)

    with tile.TileContext(nc) as tc:
        with (
            tc.tile_pool(name="consts", bufs=1) as cpool,
            tc.tile_pool(name="xin", bufs=6) as xpool,
            tc.tile_pool(name="y1", bufs=nrt + 2) as y1pool,
            tc.tile_pool(name="y1t", bufs=nwt + 4) as y1tpool,
            tc.tile_pool(name="outp", bufs=4) as outpool,
            tc.tile_pool(name="psV", bufs=3, space="PSUM") as psV,
            tc.tile_pool(name="psT", bufs=3, space="PSUM") as psT,
            tc.tile_pool(name="psH", bufs=2, space="PSUM") as psH,
        ):
            cb = _CopyBalancer(nc)
            mt_sb = cpool.tile([P, P], F32R)
            mtb_sb = cpool.tile([P, P], BF16)
            identb = cpool.tile([P, P], BF16)
            nc.sync.dma_start(mt_sb[:], mt_d[:])
            nc.sync.dma_start(mtb_sb[:], mtb_d[:])
            nc.sync.dma_start(identb[:], identb_d[:])

            for img in range(n_img):
                r0 = img * img_h

                # V-pass: f32r stream, round to bf16 on the PSUM->SBUF copy
                y1s = []
                for rt in range(nrt):
                    xt = xpool.tile([P, width], F32R)
                    nc.sync.dma_start(
                        xt[:], x_d[r0 + rt * P : r0 + (rt + 1) * P, :]
                    )
                    y1 = y1pool.tile([P, width], BF16)
                    for c in range(width // MMW):
                        ps = psV.tile([P, MMW], F32)
                        nc.tensor.matmul(
                            ps[:], mt_sb[:], xt[:, c * MMW : (c + 1) * MMW]
                        )
                        cb.copy(y1[:, c * MMW : (c + 1) * MMW], ps[:])
                    y1s.append(y1)

                # T-pass: bf16 PE transposes, 8 per PSUM bank
                y1ts = []
                for wt in range(nwt):
                    y1t = y1tpool.tile([P, img_h], dt16)
                    pst = psT.tile([P, img_h], BF16)
                    for rt in range(nrt):
                        nc.tensor.transpose(
                            pst[:, rt * P : (rt + 1) * P],
                            y1s[rt][:, wt * P : (wt + 1) * P],
                            identb[:],
                        )
                    cb.copy(y1t[:], pst[:])
                    y1ts.append(y1t)

                # fused H-pass: out chunk = (y1t_chunk)^T @ M^T  (H-major)
                for rt in range(nrt):
                    out_sb = outpool.tile([P, width], F32)
                    for half in range(nwt // 4):
                        ps = psH.tile([P, 512], F32)
                        for q in range(4):
                            wt = half * 4 + q
                            nc.tensor.matmul(
                                ps[:, q * P : (q + 1) * P],
                                y1ts[wt][:, rt * P : (rt + 1) * P],
                                mtb_sb[:],
                            )
                        cb.copy(out_sb[:, half * 512 : (half + 1) * 512], ps[:])
                    nc.sync.dma_start(
                        out_d[r0 + rt * P : r0 + (rt + 1) * P, :], out_sb[:]
                    )

    nc.compile()
    return nc


def build_nc(n_img, img_h, width, mode=MODE):
    if mode == "fused_bf16":
        return build_fused_bf16(n_img, img_h, width, BF16)
    if mode == "fused_fp16":
        return build_fused_bf16(n_img, img_h, width, FP16)
    if mode == "hybrid":
        return build_hybrid(n_img, img_h, width)
    raise ValueError(mode)


def make_mt(A: np.ndarray) -> np.ndarray:
    """M^T where M = kron(I_{128/8}, A)."""
    M = np.kron(np.eye(P // BLOCK, dtype=np.float32), A.astype(np.float32))
    return np.ascontiguousarray(M.T)


def make_inputs(mode, x_core, A):
    mt = make_mt(A)
    if mode == "fused_bf16":
        return {"x": x_core, "mtb": mt.astype(ml_dtypes.bfloat16)}
    if mode == "fused_fp16":
        return {"x": x_core, "mtb": mt.astype(np.float16)}
    if mode == "hybrid":
        return {
            "x": x_core,
            "mt": mt,
            "mtb": mt.astype(ml_dtypes.bfloat16),
            "identb": np.eye(P, dtype=ml_dtypes.bfloat16),
        }
    raise ValueError(mode)


_NC_CACHE = {}


def _get_nc(key, *args, **kwargs):
    if key not in _NC_CACHE:
        _NC_CACHE[key] = build_nc(*args, **kwargs)
    return _NC_CACHE[key]


def kernel(x: np.ndarray, A: np.ndarray) -> np.ndarray:
    x = np.asarray(x, dtype=np.float32)
    A = np.asarray(A, dtype=np.float32)
    N, C, H, W = x.shape
    assert (N, C, H, W) == (FULL_N, FULL_C, FULL_H, FULL_W), x.shape
    per = N // N_CORES

    nc = _get_nc(("full", MODE), per * C, H, W, MODE)

    in_maps = [
        make_inputs(
            MODE,
            np.ascontiguousarray(x[c * per : (c + 1) * per].reshape(per * C * H, W)),
            A,
        )
        for c in range(N_CORES)
    ]
    res = run_bass_kernel_spmd(nc, in_maps, list(range(N_CORES)))
    outs = [
        res.results[c]["out"].reshape(per, C, H, W) for c in range(N_CORES)
    ]
    return np.concatenate(outs, axis=0)
